# revision 1
# baseline (speedup 1.0000x reference)
"""Trainium2 Bass kernel: multi-head attention (B=2, S=2048, E=1024, H=16).

Sharding: 8 cores = 2 batches x 4 head-groups. Core c handles batch c//4 and
heads [4*(c%4), 4*(c%4)+4) (256 feature columns of the projections).

Per-core device program:
  - inputs: xT [E,S] (host-transposed x[b], bf16), wqT/wkT/wvT [E,256]
    (bf16 row-slices of Wq/Wk/Wv), woT [256,E] (fp32r column slice of Wo).
  - qT,kT [256,S] = (x @ W^T)^T per head-group, computed directly in [f,s]
    layout; v [S,256] in [s,f] layout with a ones column appended per head.
  - per (head-pair, qi-chunk): scores^T tiles [128 kj, 512 qi] on PE, exp on
    ACT (sm_scale folded into the activation scale), attn@v accumulated on PE
    with the ones column producing the softmax denominator in partition 64,
    then reciprocal + partition-broadcast + multiply to normalize; output
    kept in [f, s] layout for the output projection.
  - out_partial [S,E] (bf16) = o @ Wo^T column-slice; host sums 4 partials
    per batch in f32 and adds bo.

Scheduling (this is where the performance is):
  - The kernel is ~85% PE-bound; the attention inner loop is paced by the
    ACT exp (~1.1us per key-tile vs 854ns of PE work), so every other PE op
    (q/k/v projections for later chunks, output projections of finished
    chunks) is emitted as a pump()-driven "filler" inside the attention
    loops' idle slots, in strict data-deadline order.
  - The two head-pairs' chunks interleave (0,0),(0,1),(1,0),(0,2),(1,1),
    (0,3),(1,2),(1,3) so out-proj fillers unlock throughout the span.
  - DMA ordering minimizes time-to-first-matmul (~625ns HWDGE + 650ns DGE +
    900ns sem-prop fixed cost per dma_start; one merged x tile per chunk).
  - The tail chunk normalizes straight out of PSUM across three parallel
    engine streams (DVE reciprocal, Pool broadcast+mul, ACT drain+copy) and
    splits its output DMAs per 512-column half to shorten the end chain.
"""

import numpy as np

import concourse.tile as tile
import concourse.mybir as mybir
from concourse import bacc
from concourse.bass_utils import run_bass_kernel_spmd

B, S, E, H, D = 2, 2048, 1024, 16, 64
NCORES = 8
GPB = NCORES // B      # head-groups (cores) per batch = 4
HPC = H // GPB         # heads per core = 4
FPC = HPC * D          # feature cols per core = 256
SM = float(D) ** -0.5  # softmax scale

F32 = mybir.dt.float32
F32R = mybir.dt.float32r
BF16 = mybir.dt.bfloat16

P = 128
NE = E // P            # 8 e-tiles
NST = S // P           # 16 s-tiles (key tiles)
NQ = 4                 # qi chunks
QC = S // NQ           # 512
KTG = 2                # k-tiles per psum/exp group
NKG = NST // KTG       # 8 groups
FT = FPC // P          # 2 f-tiles per core


def _round_fp32r(a: np.ndarray) -> np.ndarray:
    """Round fp32 to the fp32r encoding (RNE to 12-bit mantissa)."""
    u = np.ascontiguousarray(a, dtype=np.float32).view(np.uint32)
    lo = u & np.uint32(0xFFF)
    base = u & ~np.uint32(0xFFF)
    rup = (lo > 0x800) | ((lo == 0x800) & (((base >> np.uint32(12)) & np.uint32(1)) == 1))
    out = base + (rup.astype(np.uint32) << np.uint32(12))
    return out.view(np.float32)


def _build():
    nc = bacc.Bacc("TRN2", target_bir_lowering=False, debug=False)

    xT_d = nc.dram_tensor("xT", [E, S], BF16, kind="ExternalInput")
    wq_d = nc.dram_tensor("wqT", [E, FPC], BF16, kind="ExternalInput")
    wk_d = nc.dram_tensor("wkT", [E, FPC], BF16, kind="ExternalInput")
    wv_d = nc.dram_tensor("wvT", [E, FPC], BF16, kind="ExternalInput")
    wo_d = nc.dram_tensor("woT", [FPC, E], F32R, kind="ExternalInput")
    ones_lhs_d = nc.dram_tensor("ones_lhs", [1, D], F32R, kind="ExternalInput")
    ones_col_d = nc.dram_tensor("ones_col", [P, HPC, 1], F32R, kind="ExternalInput")
    out_d = nc.dram_tensor("out", [S, E], BF16, kind="ExternalOutput")

    with tile.TileContext(nc) as tc:
        with (
            tc.tile_pool(name="wpool", bufs=1) as wpool,
            tc.tile_pool(name="xpool", bufs=1) as xpool,
            tc.tile_pool(name="qkpool", bufs=1) as qkpool,
            tc.tile_pool(name="vpool", bufs=1) as vpool,
            tc.tile_pool(name="opool", bufs=1) as opool,
            tc.tile_pool(name="epool", bufs=14) as epool,
            tc.tile_pool(name="spool", bufs=2) as spool,
            tc.tile_pool(name="outpool", bufs=4) as outpool,
            tc.tile_pool(name="pspool", bufs=2, space="PSUM") as pspool,
            tc.tile_pool(name="popool", bufs=2, space="PSUM") as popool,
            tc.tile_pool(name="oaccpool", bufs=2, space="PSUM") as oaccpool,
        ):
            # ---- weights / constants -------------------------------------
            wq = wpool.tile([P, NE, FPC], BF16, name="wq")
            wk = wpool.tile([P, NE, FPC], BF16, name="wk")
            wv = wpool.tile([P, NE, FPC], BF16, name="wv")
            wo = wpool.tile([P, FT, E], F32R, name="wo")
            ones = wpool.tile([1, D], F32R, name="ones")
            wk_r = wk_d.ap().rearrange("(t p) f -> p t f", p=P)
            wq_r = wq_d.ap().rearrange("(t p) f -> p t f", p=P)
            wv_r = wv_d.ap().rearrange("(t p) f -> p t f", p=P)

            # ---- x^T (chunk-major DMA so compute starts early) -----------
            xT_r = xT_d.ap().rearrange("(t p) s -> p t s", p=P)
            # x in bf16: halves the startup-critical HBM traffic; the ~0.2%
            # relative quantization is well inside the 2e-2 tolerance. One
            # [P, NE, S] tile so a whole query-chunk's worth of x moves in a
            # single dma_start -- each dma_start costs ~625ns of serialized
            # HWDGE + 650ns DGE + 900ns sem-prop, so instruction count, not
            # bytes, dominated the old per-et startup stream.
            xtile = xpool.tile([P, NE, S], BF16, name="xtile")
            xts = [xtile[:, et, :] for et in range(NE)]
            # Chunk 0 is latency-critical: stage it so the k0 accumulation
            # (et order) starts after ~600KB instead of the full 2MB.
            nc.sync.dma_start(out=wk[:, 0:1, 0:P], in_=wk_r[:, 0:1, 0:P])
            nc.sync.dma_start(out=xtile[:, 0:1, 0:QC], in_=xT_r[:, 0:1, 0:QC])
            nc.sync.dma_start(out=wk[:, 1:NE, 0:P], in_=wk_r[:, 1:NE, 0:P])
            nc.sync.dma_start(out=xtile[:, 1:2, 0:QC], in_=xT_r[:, 1:2, 0:QC])
            nc.sync.dma_start(out=xtile[:, 2:4, 0:QC], in_=xT_r[:, 2:4, 0:QC])
            nc.sync.dma_start(out=xtile[:, 4:6, 0:QC], in_=xT_r[:, 4:6, 0:QC])
            nc.sync.dma_start(out=xtile[:, 6:NE, 0:QC], in_=xT_r[:, 6:NE, 0:QC])
            nc.sync.dma_start(out=wq[:, :, 0:P], in_=wq_r[:, :, 0:P])
            nc.sync.dma_start(out=wv[:, 0:4, :], in_=wv_r[:, 0:4, :])
            nc.sync.dma_start(out=wv[:, 4:NE, :], in_=wv_r[:, 4:NE, :])
            nc.sync.dma_start(out=ones, in_=ones_lhs_d.ap())
            for cq in range(1, NQ):
                csl = slice(cq * QC, (cq + 1) * QC)
                nc.sync.dma_start(out=xtile[:, :, csl], in_=xT_r[:, :, csl])
                if cq == 2:
                    nc.sync.dma_start(out=wk[:, :, P:FPC], in_=wk_r[:, :, P:FPC])
                    nc.sync.dma_start(out=wq[:, :, P:FPC], in_=wq_r[:, :, P:FPC])

            nc.sync.dma_start(out=wo, in_=wo_d.ap().rearrange("(t p) g -> p t g", p=P))

            # ---- v projection: v[s, f] with ones col per head ------------
            v_tiles = [
                vpool.tile([P, HPC, D + 1], F32R, name=f"v{st}", tag=f"v{st}")
                for st in range(NST)
            ]

            def proj_v(st):
                vt = v_tiles[st]
                nc.sync.dma_start(out=vt[:, :, D : D + 1], in_=ones_col_d.ap())
                ps_v = popool.tile([P, FPC], F32, name="ps_v", tag="po")
                for et in range(NE):
                    nc.tensor.matmul(
                        ps_v,
                        xts[et][:, st * P : (st + 1) * P],
                        wv[:, et, :],
                        start=(et == 0),
                        stop=(et == NE - 1),
                    )
                nc.vector.tensor_copy(
                    vt[:, :, 0:D], ps_v.rearrange("p (h d) -> p h d", d=D)
                )

            # ---- q^T / k^T projections: [f, s] ---------------------------
            def proj_T(w_tile, dst_tiles, which, ft, cq):
                ps = popool.tile([P, QC], F32, name=f"ps_{which}", tag="po")
                for et in range(NE):
                    nc.tensor.matmul(
                        ps,
                        w_tile[:, et, ft * P : (ft + 1) * P],
                        xts[et][:, cq * QC : (cq + 1) * QC],
                        start=(et == 0),
                        stop=(et == NE - 1),
                    )
                nc.vector.tensor_copy(
                    dst_tiles[ft][:, cq * QC : (cq + 1) * QC], ps
                )

            kts = [qkpool.tile([P, S], F32R, name=f"kt{ft}", tag=f"kt{ft}") for ft in range(FT)]
            qts = [qkpool.tile([P, S], F32R, name=f"qt{ft}", tag=f"qt{ft}") for ft in range(FT)]
            ots = [opool.tile([P, S], F32R, name=f"ot{ft}", tag=f"ot{ft}") for ft in range(FT)]

            # Filler machinery: generators that emit one PE-side instruction
            # per next() call. attn_core drains a couple of units after each
            # kt step, so independent matmul work lands inside the PE idle
            # gaps of the ACT-bound attention inner loop instead of between
            # cores (the PE executes its stream in order).
            from collections import deque

            fillers = deque()

            def pump(n):
                for _ in range(n):
                    while fillers:
                        try:
                            next(fillers[0])
                            break
                        except StopIteration:
                            fillers.popleft()
                    else:
                        return

            def attn_core(pair, cq, per_kt=2):
                """Heads 2*pair, 2*pair+1 for query chunk cq; the two heads'
                score matmuls run concurrently on PE row-groups 0-63/64-127.
                Returns the two accumulation psum tiles (rows 0..63 =
                sum(exp*v), row 64 = sum(exp))."""
                ft = pair
                csl = slice(cq * QC, (cq + 1) * QC)
                ps_o = [
                    oaccpool.tile([D + 1, QC], F32, name=f"ps_o{s}", tag="oacc")
                    for s in range(2)
                ]
                def attnv(kt, et_t):
                    for sub in range(2):
                        nc.tensor.matmul(
                            ps_o[sub],
                            v_tiles[kt][:, 2 * pair + sub, :],
                            et_t[:, sub, :],
                            start=(kt == 0),
                            stop=(kt == NST - 1),
                        )

                def scores(kt):
                    et_t = epool.tile([P, 2, QC], F32R, name="et_t", tag="et_t")
                    ps_s = pspool.tile([P, 2, QC], F32, name="ps_s", tag="ps_s")
                    for sub in range(2):
                        lo, hi = sub * D, (sub + 1) * D
                        nc.tensor.matmul(
                            ps_s[:, sub, :],
                            kts[ft][lo:hi, kt * P : (kt + 1) * P],
                            qts[ft][lo:hi, csl],
                            start=True,
                            stop=True,
                        )
                    nc.scalar.activation(
                        out=et_t,
                        in_=ps_s,
                        func=mybir.ActivationFunctionType.Exp,
                        scale=SM,
                    )
                    return et_t

                # kt0's attn@v (start=True) waits for the previous chunk's
                # accumulator drain; emitting it after kt1's scores hides
                # that latency behind useful PE work.
                et0 = scores(0)
                et1 = scores(1)
                attnv(0, et0)
                attnv(1, et1)
                pump(per_kt)
                for kt in range(2, NST):
                    et_t = scores(kt)
                    attnv(kt, et_t)
                    pump(per_kt)
                return ps_o

            def attn_drain(ps_o):
                """Copy both accumulators (incl. the sum row) to SBUF right
                away so the psum slots free early."""
                o_full = []
                for sub in range(2):
                    of = epool.tile([D + 1, QC], F32, name="o_hat", tag="o_hat", bufs=4)
                    nc.vector.tensor_copy(of, ps_o[sub])
                    o_full.append(of)
                return o_full

            def bcast_recip(o_full):
                """Reciprocal of each sum row, partition-broadcast on the
                (otherwise idle) GPSIMD engine. No PE/ACT work."""
                bcs = []
                for sub in range(2):
                    rec = spool.tile([1, QC], F32, name="rec", tag="rec", bufs=1)
                    nc.vector.reciprocal(rec, o_full[sub][D : D + 1, :])
                    bc = spool.tile([D, QC], F32, name="bc", tag="bc", bufs=4)
                    nc.gpsimd.partition_broadcast(bc, rec)
                    bcs.append(bc)
                return bcs

            def attn_finish(pair, cq, o_full):
                """Normalize a pair-0 chunk (full-width multiply)."""
                csl = slice(cq * QC, (cq + 1) * QC)
                bcs = bcast_recip(o_full)
                for sub in range(2):
                    lo, hi = sub * D, (sub + 1) * D
                    nc.vector.tensor_mul(
                        ots[pair][lo:hi, csl], o_full[sub][0:D, :], bcs[sub]
                    )

            def finish_outproj_units(cq, o_full, bcs, tail=False):
                """Pair-1 normalize pipelined with the output projection at
                s-tile granularity (shortens the kernel tail). In the tail
                the PSUM->SBUF copies ride the idle ACT engine instead of
                DVE."""
                # Normalize all four s-tiles first so the muls get a pump
                # head-start over their out-proj matmuls; alternate DVE and
                # the otherwise-idle Pool engine. Pool cannot read PSUM, so
                # in the tail sub0 (PSUM bc) goes to DVE and sub1 (SBUF bc
                # via partition_broadcast) to Pool -- the two streams run
                # concurrently.
                for sti in range(NQ):
                    st = cq * NQ + sti
                    ssl = slice(sti * P, (sti + 1) * P)
                    for sub in range(2):
                        lo, hi = sub * D, (sub + 1) * D
                        if tail:
                            # sub1's o stays in PSUM -> DVE only. Sub0 (ACT-
                            # drained to SBUF) runs the first s-tile on DVE
                            # too (its reciprocals are done first); Pool takes
                            # the later ones behind its partition_broadcasts.
                            eng = (
                                nc.gpsimd if sub == 0 and sti > 0 else nc.vector
                            )
                        else:
                            # Pool also runs the chunk's partition_broadcasts;
                            # give it only 2 of the 8 muls so it doesn't gate
                            # the out-proj matmuls.
                            eng = (
                                nc.gpsimd
                                if sub == 1 and sti % 2 == 1
                                else nc.vector
                            )
                        eng.tensor_mul(
                            ots[1][lo:hi, st * P : (st + 1) * P],
                            o_full[sub][0:D, ssl],
                            bcs[sub][:, ssl],
                        )
                    yield
                for sti in range(NQ):
                    st = cq * NQ + sti
                    out_sb = outpool.tile([P, E], BF16, name="out_sb", tag="out_sb")
                    for gc in range(2):
                        if tail and sti % 2 == 1:
                            # the score psum (ps_s) is free once the last exp
                            # retired; rotating the tail's out-proj psum over
                            # both pools doubles the slots so the PSUM->SBUF
                            # copies stop gating the matmuls.
                            ps_out = pspool.tile([P, QC], F32, name="ps_out", tag="ps_s")
                        else:
                            ps_out = popool.tile([P, QC], F32, name="ps_out", tag="po")
                        for ft in range(FT):
                            nc.tensor.matmul(
                                ps_out,
                                ots[ft][:, st * P : (st + 1) * P],
                                wo[:, ft, gc * QC : (gc + 1) * QC],
                                start=(ft == 0),
                                stop=(ft == FT - 1),
                            )
                            yield
                        osl = slice(gc * QC, (gc + 1) * QC)
                        if tail:
                            # alternate engines so the two copies overlap.
                            # Each dma_start costs ~625ns of serialized HWDGE,
                            # so only the LAST s-tile (whose final DMA is the
                            # kernel tail) gets per-half DMAs; the others use
                            # one full-row DMA below.
                            if gc == 0:
                                nc.scalar.activation(
                                    out=out_sb[:, osl],
                                    in_=ps_out,
                                    func=mybir.ActivationFunctionType.Copy,
                                )
                            elif sti == NQ - 1:
                                # the very last copy: split across ACT and
                                # DVE so the final DMA's wait halves.
                                nc.scalar.activation(
                                    out=out_sb[:, QC : QC + QC // 2],
                                    in_=ps_out[:, 0 : QC // 2],
                                    func=mybir.ActivationFunctionType.Copy,
                                )
                                nc.vector.tensor_copy(
                                    out_sb[:, QC + QC // 2 : E],
                                    ps_out[:, QC // 2 : QC],
                                )
                            else:
                                nc.vector.tensor_copy(out_sb[:, osl], ps_out)
                            if sti == NQ - 1:
                                nc.sync.dma_start(
                                    out=out_d.ap()[st * P : (st + 1) * P, osl],
                                    in_=out_sb[:, osl],
                                )
                        else:
                            nc.vector.tensor_copy(out_sb[:, osl], ps_out)
                        yield
                    if not (tail and sti == NQ - 1):
                        nc.sync.dma_start(
                            out=out_d.ap()[st * P : (st + 1) * P, :], in_=out_sb
                        )

            # Emission order = scheduler priority. Attention cores are
            # emitted right after the projections of their own chunk, so the
            # first exp fires as soon as chunk-0 data exists; later-chunk
            # projections backfill PE whenever attention is dep-blocked.
            def proj_v_units(sts):
                """Deferred v projections, pumped as attention fillers: the
                attn@v for key-tile st only needs v[st] at inner step st, so
                the tail tiles can ride the ACT-bound attention loop instead
                of extending the serial projection phase."""
                for st in sts:
                    vt = v_tiles[st]
                    nc.sync.dma_start(out=vt[:, :, D : D + 1], in_=ones_col_d.ap())
                    ps_v = popool.tile([P, FPC], F32, name="ps_v", tag="po")
                    for et in range(NE):
                        nc.tensor.matmul(
                            ps_v,
                            xts[et][:, st * P : (st + 1) * P],
                            wv[:, et, :],
                            start=(et == 0),
                            stop=(et == NE - 1),
                        )
                        yield
                    nc.vector.tensor_copy(
                        vt[:, :, 0:D], ps_v.rearrange("p (h d) -> p h d", d=D)
                    )
                    yield

            def proj_unit(w_tile, dst_tiles, which, ft, cq):
                """One deferred projection unit: 8 accumulating matmuls plus
                the PSUM->SBUF copy, yielding per instruction."""
                ps = popool.tile([P, QC], F32, name=f"ps_{which}", tag="po")
                for et in range(NE):
                    nc.tensor.matmul(
                        ps,
                        w_tile[:, et, ft * P : (ft + 1) * P],
                        xts[et][:, cq * QC : (cq + 1) * QC],
                        start=(et == 0),
                        stop=(et == NE - 1),
                    )
                    yield
                nc.vector.tensor_copy(
                    dst_tiles[ft][:, cq * QC : (cq + 1) * QC], ps
                )
                yield

            N_VDEFER = 4
            for cq in range(NQ):
                proj_T(wk, kts, "k0", 0, cq)
                if cq == 0:
                    proj_T(wq, qts, "q0", 0, 0)
                for st in range(cq * NQ, (cq + 1) * NQ):
                    if st < NST - N_VDEFER:
                        proj_v(st)

            def deferred_proj_units():
                """All deferred projections in deadline order for the
                interleaved chunk schedule below: v tiles by (0,0) kt12-15;
                q0-cq by phase (0,cq); k1-c0 + q1-c0 by phase (1,0) with
                k1-cq first read at its inner step 4cq; q1-cq by (1,cq)."""
                yield from proj_v_units(range(NST - N_VDEFER, NST))
                yield from proj_unit(wq, qts, "q0", 0, 1)
                yield from proj_unit(wk, kts, "k1", 1, 0)
                yield from proj_unit(wq, qts, "q1", 1, 0)
                yield from proj_unit(wk, kts, "k1", 1, 1)
                yield from proj_unit(wk, kts, "k1", 1, 2)
                yield from proj_unit(wk, kts, "k1", 1, 3)
                yield from proj_unit(wq, qts, "q0", 0, 2)
                yield from proj_unit(wq, qts, "q1", 1, 1)
                yield from proj_unit(wq, qts, "q0", 0, 3)
                yield from proj_unit(wq, qts, "q1", 1, 2)
                yield from proj_unit(wq, qts, "q1", 1, 3)

            # Deferred work must be queued BEFORE attn_core(0, 0) emits: its
            # pump() calls drain the deque as chunk 0 unrolls, and its
            # kt=12..15 attn@v matmuls consume the deferred v tiles.
            fillers.append(deferred_proj_units())

            # Interleave the two head-pairs at chunk granularity: out-proj
            # fillers for chunk cq unlock right after (1,cq), spreading PE
            # backfill across the whole attention span instead of bunching
            # it at the end.
            SCHED = [(0, 0), (0, 1), (1, 0), (0, 2), (1, 1), (0, 3), (1, 2), (1, 3)]
            PER_KT = {(0, 0): 3, (0, 1): 2, (1, 1): 1, (0, 3): 2}
            if True:
                for pair, cq in SCHED:
                    tail = pair == 1 and cq == NQ - 1
                    ps_o = attn_core(pair, cq, per_kt=PER_KT.get((pair, cq), 2 if pair else 1))
                    if pair == 0:
                        of = attn_drain(ps_o)
                        attn_finish(pair, cq, of)
                    elif not tail:
                        of = attn_drain(ps_o)
                        bcs = bcast_recip(of)
                        fillers.append(finish_outproj_units(cq, of, bcs))
                    else:
                        # tail chunk: normalization is the critical path.
                        # Reciprocals read the denominator rows straight from
                        # PSUM (in halves so the first s-tile unblocks
                        # early); Pool broadcasts them to SBUF. Sub0's o is
                        # drained to SBUF by the otherwise-idle ACT engine so
                        # its muls are all-SBUF and can run on Pool; sub1's
                        # muls read PSUM directly on DVE. Three parallel
                        # engine streams instead of one DVE chain.
                        H0 = slice(0, P)
                        H1 = slice(P, QC)
                        recs, bcs = [], []
                        for sub in range(2):
                            recs.append(spool.tile(
                                [1, QC], F32, name=f"rec{sub}", tag=f"rec{sub}", bufs=1
                            ))
                            bcs.append(spool.tile(
                                [D, QC], F32, name=f"bct{sub}", tag=f"bct{sub}", bufs=1
                            ))
                        # H0 for both subs first: the first s-tile's muls only
                        # need the leading 128 columns, so they unblock after
                        # two short reciprocals + broadcasts.
                        for hs in (H0, H1):
                            for sub in range(2):
                                nc.vector.reciprocal(
                                    recs[sub][:, hs], ps_o[sub][D : D + 1, hs]
                                )
                            for sub in range(2):
                                nc.gpsimd.partition_broadcast(
                                    bcs[sub][:, hs], recs[sub][:, hs]
                                )
                        of0 = epool.tile([D + 1, QC], F32, name="o_hat", tag="o_hat", bufs=4)
                        nc.scalar.copy(of0[:, H0], ps_o[0][:, H0])
                        nc.scalar.copy(of0[:, H1], ps_o[0][:, H1])
                        fillers.appendleft(
                            finish_outproj_units(cq, [of0, ps_o[1]], bcs, tail=True)
                        )
            # drain remaining fillers (the last chunk's output projection)
            while fillers:
                pump(64)

    nc.compile()
    return nc


_NC_CACHE = None


def _get_nc():
    global _NC_CACHE
    if _NC_CACHE is None:
        _NC_CACHE = _build()
    return _NC_CACHE


def make_in_maps(x, Wq, Wk, Wv, Wo):
    import ml_dtypes

    in_maps = []
    xTs = [
        np.ascontiguousarray(x[b].T).astype(ml_dtypes.bfloat16) for b in range(B)
    ]
    for c in range(NCORES):
        b, hg = c // GPB, c % GPB
        fsl = slice(hg * FPC, (hg + 1) * FPC)
        in_maps.append({
            "xT": xTs[b],
            "wqT": np.ascontiguousarray(Wq[fsl, :].T).astype(ml_dtypes.bfloat16),
            "wkT": np.ascontiguousarray(Wk[fsl, :].T).astype(ml_dtypes.bfloat16),
            "wvT": np.ascontiguousarray(Wv[fsl, :].T).astype(ml_dtypes.bfloat16),
            "woT": _round_fp32r(Wo[:, fsl].T),
            "ones_lhs": np.ones((1, D), dtype=np.float32),
            "ones_col": np.ones((P, HPC, 1), dtype=np.float32),
        })
    return in_maps


def kernel(x, Wq, bq, Wk, bk, Wv, bv, Wo, bo):
    x = np.asarray(x, dtype=np.float32)
    Wq, Wk, Wv, Wo = (np.asarray(a, dtype=np.float32) for a in (Wq, Wk, Wv, Wo))
    bq, bk, bv, bo = (np.asarray(a, dtype=np.float32) for a in (bq, bk, bv, bo))
    if np.any(bq) or np.any(bk) or np.any(bv):
        # fall back: fold nonzero projection biases into an augmented input
        # row is not implemented; biases are zero for this problem spec.
        raise NotImplementedError("nonzero projection biases not supported")

    nc = _get_nc()
    in_maps = make_in_maps(x, Wq, Wk, Wv, Wo)
    res = run_bass_kernel_spmd(nc, in_maps, core_ids=list(range(NCORES)))
    out = np.empty((B, S, E), dtype=np.float32)
    for b in range(B):
        acc = res.results[b * GPB]["out"].astype(np.float32)
        for hg in range(1, GPB):
            acc = acc + res.results[b * GPB + hg]["out"].astype(np.float32)
        out[b] = acc
    out += bo[None, None, :]
    return out

